# revision 3
# baseline (speedup 1.0000x reference)
"""Multi-head cross-attention (B=2, N=1024, L=4096, D=1024, H=16) on 8 trn2
NeuronCores.

Sharding: batch x head-group data/tensor parallel. Core c handles batch
c//4 and heads 4*(c%4) .. 4*(c%4)+3 (weight columns sliced per head group,
Wo row-sliced; partial outputs summed on the host during unsharding).

Per-core device program (all matmuls in fp32r at full PE rate):
  qT/kT = W.T @ x.T    (channels on partitions, head pairs stacked 64+64)
  v     = x @ Wv       (keys on partitions) augmented with a ones column and
                       pre-multiplied by the pad-keep mask (this implements
                       the padding mask exactly: masked keys contribute to
                       neither numerator nor denominator)
  per (query-block, head-pair, keytile):
     sT[keys,q] = kT.T @ qT   (two row-paired K=64 matmuls)
     pT = exp(0.125 * sT)     (one ACT op over both heads' banks)
     oT_aug[65,q] += v_aug.T @ pT   (PSUM accumulation; row 64 = denominator)
  out_part = (oT/denom).T @ Wo_slice   (+ q/k/v biases via K=1 matmuls)
"""
import sys

sys.path.insert(0, "/opt/trn_rl_repo")

import numpy as np

import concourse.bass as bass
import concourse.tile as tile
from concourse import bacc, mybir
from concourse.bass_utils import run_bass_kernel_spmd

dt = mybir.dt
ts = bass.ts

B, N, L, D = 2, 1024, 4096, 1024
H, DH = 16, 64
HC = 4            # heads per core
CS = HC * DH      # 256 channel slice per core
SCALE = DH ** -0.5
N_CORES = 8
QB, KB = 2, 8     # query blocks of 512, key blocks of 512
DQC = 8           # contraction chunks of 128
KT = 32           # keytiles of 128

TRACE = False
LAST_EXEC_NS = None
_cache = {}


def _build():
    nc = bacc.Bacc("TRN2", target_bir_lowering=False, debug=False,
                   num_devices=N_CORES)

    xTq = nc.dram_tensor("xTq", [D, N], dt.float32, kind="ExternalInput").ap()
    xTkv = nc.dram_tensor("xTkv", [D, L], dt.float32, kind="ExternalInput").ap()
    wq = nc.dram_tensor("wq", [D, CS], dt.float32, kind="ExternalInput").ap()
    wk = nc.dram_tensor("wk", [D, CS], dt.float32, kind="ExternalInput").ap()
    wv = nc.dram_tensor("wv", [D, CS], dt.float32, kind="ExternalInput").ap()
    wo = nc.dram_tensor("wo", [CS, D], dt.float32, kind="ExternalInput").ap()
    bqv = nc.dram_tensor("bqv", [1, CS], dt.float32, kind="ExternalInput").ap()
    bkv = nc.dram_tensor("bkv", [1, CS], dt.float32, kind="ExternalInput").ap()
    bvv = nc.dram_tensor("bvv", [1, CS], dt.float32, kind="ExternalInput").ap()
    keep = nc.dram_tensor("keep", [128, KT, HC], dt.float32,
                          kind="ExternalInput").ap()
    out = nc.dram_tensor("out", [N, D], dt.float32, kind="ExternalOutput").ap()

    with tile.TileContext(nc) as tc:
        _emit(nc, tc, xTq, xTkv, wq, wk, wv, wo, bqv, bkv, bvv, keep, out)
    nc.compile()
    return nc


def _emit(nc, tc, xTq, xTkv, wq, wk, wv, wo, bqv, bkv, bvv, keep, out):
    import contextlib

    ctx = contextlib.ExitStack()
    with ctx:
        persist = ctx.enter_context(tc.tile_pool(name="persist", bufs=1))
        xstage = ctx.enter_context(tc.tile_pool(name="xstage", bufs=3))
        xr_pool = ctx.enter_context(tc.tile_pool(name="xr", bufs=10))
        pT_pool = ctx.enter_context(tc.tile_pool(name="pT", bufs=3))
        rb_pool = ctx.enter_context(tc.tile_pool(name="rbs", bufs=2))
        outsb_pool = ctx.enter_context(tc.tile_pool(name="outsb", bufs=3))
        psA = ctx.enter_context(tc.tile_pool(name="psA", bufs=1, space="PSUM"))
        psS = ctx.enter_context(tc.tile_pool(name="psS", bufs=2, space="PSUM"))
        psO = ctx.enter_context(tc.tile_pool(name="psO", bufs=1, space="PSUM"))
        lp = nc.allow_low_precision(reason="fp32r/bf16 attention internals")
        lp.__enter__()

        # ---- weights / biases / constants -------------------------------
        def load_round(name, src, shape):
            f = xstage.tile(shape, dt.float32, tag="wstage", name=f"{name}_f")
            nc.sync.dma_start(f[:], src)
            r = persist.tile(shape, dt.float32r, tag=name, name=name)
            nc.vector.tensor_copy(r[:], f[:])
            return r

        wq_r = [load_round(f"wq{i}", wq[ts(i, 128), :], [128, CS]) for i in range(DQC)]
        wk_r = [load_round(f"wk{i}", wk[ts(i, 128), :], [128, CS]) for i in range(DQC)]
        wv_r = [load_round(f"wv{i}", wv[ts(i, 128), :], [128, CS]) for i in range(DQC)]
        wo_r = [load_round(f"wo{i}", wo[ts(i, 128), :], [128, D]) for i in range(2)]
        bq_r = load_round("bqr", bqv, [1, CS])
        bk_r = load_round("bkr", bkv, [1, CS])
        bv_r = load_round("bvr", bvv, [1, CS])
        keep_f = persist.tile([128, KT, HC], dt.float32, tag="keepf", name="keep_f")
        nc.sync.dma_start(keep_f[:], keep)

        ones_f = xstage.tile([1, 512], dt.float32, tag="wstage", name="ones_f")
        nc.vector.memset(ones_f[:], 1.0)
        ones512_r = persist.tile([1, 512], dt.float32r, tag="o512", name="ones512_r")
        nc.vector.tensor_copy(ones512_r[:], ones_f[:])
        ones128_r = persist.tile([1, 128], dt.float32r, tag="o128", name="ones128_r")
        nc.vector.tensor_copy(ones128_r[:], ones_f[:, 0:128])

        # ---- persistent activation tiles --------------------------------
        qT_sb = [persist.tile([128, N], dt.float32r, tag=f"qT{cc}", name=f"qT{cc}")
                 for cc in range(2)]
        kT_sb = [[persist.tile([128, 512], dt.float32r, tag=f"kT{cc}_{kb}",
                               name=f"kT{cc}_{kb}") for kb in range(KB)]
                 for cc in range(2)]
        va_sb = [persist.tile([128, HC, 65], dt.float32r, tag=f"va{kt}",
                              name=f"va{kt}") for kt in range(KT)]
        onT_sb = [persist.tile([128, N], dt.float32r, tag=f"onT{cc}",
                               name=f"onT{cc}") for cc in range(2)]

        # ---- Q projection ----------------------------------------------
        for qb in range(QB):
            qp = psA.tile([128, 1024], dt.float32, tag="psA", name=f"qp{qb}")
            for dq in range(DQC):
                xf = xstage.tile([128, 512], dt.float32, tag="xs", name=f"xfq{qb}_{dq}")
                nc.sync.dma_start(xf[:], xTq[ts(dq, 128), ts(qb, 512)])
                xr = xr_pool.tile([128, 512], dt.float32r, tag="xr", name=f"xrq{qb}_{dq}")
                nc.vector.tensor_copy(xr[:], xf[:])
                for cc in range(2):
                    nc.tensor.matmul(qp[:, ts(cc, 512)], wq_r[dq][:, ts(cc, 128)],
                                     xr[:], start=(dq == 0), stop=False)
            for cc in range(2):
                nc.tensor.matmul(qp[:, ts(cc, 512)], bq_r[:, ts(cc, 128)],
                                 ones512_r[:], start=False, stop=True)
                nc.vector.tensor_copy(qT_sb[cc][:, ts(qb, 512)], qp[:, ts(cc, 512)])

        # ---- K/V projections interleaved with attention on (qb0, hp0) ---
        oPs = {}

        def open_oP(qb, hp):
            oPs[(qb, hp)] = [
                psO.tile([128, 512], dt.float32, tag=f"oP{h}", name=f"oP{qb}{hp}{h}")
                for h in range(2)
            ]

        def attn_kt(qb, hp, kt):
            kb, kti = kt // 4, kt % 4
            sp = psS.tile([128, 1024], dt.float32, tag="sp", name=f"sp{qb}{hp}{kt}")
            for h in range(2):
                nc.tensor.matmul(
                    sp[:, ts(h, 512)],
                    kT_sb[hp][kb][ts(h, 64), ts(kti, 128)],
                    qT_sb[hp][ts(h, 64), ts(qb, 512)],
                    start=True, stop=True,
                )
            pT = pT_pool.tile([128, 1024], dt.float32r, tag="pT", name=f"pT{qb}{hp}{kt}")
            nc.scalar.activation(pT[:], sp[:], mybir.ActivationFunctionType.Exp,
                                 scale=float(SCALE))
            oP = oPs[(qb, hp)]
            for h in range(2):
                nc.tensor.matmul(
                    oP[h][0:65, :], va_sb[kt][:, hp * 2 + h, :], pT[:, ts(h, 512)],
                    start=(kt == 0), stop=(kt == KT - 1),
                )

        def attn_norm(qb, hp):
            oP = oPs.pop((qb, hp))
            for h in range(2):
                rd = rb_pool.tile([1, 512], dt.float32r, tag="rd", name=f"rd{qb}{hp}{h}")
                nc.vector.reciprocal(rd[:], oP[h][64:65, :])
                rb = psA.tile([128, 512], dt.float32, tag="psA", name=f"rb{qb}{hp}{h}")
                nc.tensor.matmul(rb[:, :], ones128_r[:], rd[:], start=True, stop=True)
                rb_sb = rb_pool.tile([128, 512], dt.float32, tag="rbs",
                                     name=f"rbs{qb}{hp}{h}")
                nc.vector.tensor_copy(rb_sb[:], rb[:])
                nc.vector.tensor_mul(onT_sb[hp][ts(h, 64), ts(qb, 512)],
                                     oP[h][0:64, :], rb_sb[0:64, :])

        open_oP(0, 0)
        for kb in range(KB):
            # K projection for this key block
            kp = psA.tile([128, 1024], dt.float32, tag="psA", name=f"kp{kb}")
            xrs = []
            for dq in range(DQC):
                xf = xstage.tile([128, 512], dt.float32, tag="xs", name=f"xfk{kb}_{dq}")
                nc.sync.dma_start(xf[:], xTkv[ts(dq, 128), ts(kb, 512)])
                xr = xr_pool.tile([128, 512], dt.float32r, tag="xr", name=f"xrk{kb}_{dq}")
                nc.vector.tensor_copy(xr[:], xf[:])
                xrs.append(xr)
                for cc in range(2):
                    nc.tensor.matmul(kp[:, ts(cc, 512)], wk_r[dq][:, ts(cc, 128)],
                                     xr[:], start=(dq == 0), stop=False)
            for cc in range(2):
                nc.tensor.matmul(kp[:, ts(cc, 512)], bk_r[:, ts(cc, 128)],
                                 ones512_r[:], start=False, stop=True)
                nc.vector.tensor_copy(kT_sb[cc][kb][:], kp[:, ts(cc, 512)])

            # V projection for this key block (4 keytiles)
            vp = psA.tile([128, 1024], dt.float32, tag="psA", name=f"vp{kb}")
            for dq in range(DQC):
                for t in range(4):
                    # start clears has_written for the whole 2KB psum bank, so
                    # only the first matmul touching each bank may set it
                    nc.tensor.matmul(vp[:, ts(t, 256)], xrs[dq][:, ts(t, 128)],
                                     wv_r[dq][:],
                                     start=(dq == 0 and t % 2 == 0), stop=False)
            for t in range(4):
                nc.tensor.matmul(vp[:, ts(t, 256)], ones128_r[:], bv_r[:],
                                 start=False, stop=True)
            for t in range(4):
                kt = kb * 4 + t
                va = va_sb[kt]
                src = vp[:, ts(t, 256)].rearrange("p (h c) -> p h c", h=HC)
                nc.vector.tensor_scalar_mul(va[:, :, 0:64], src,
                                            keep_f[:, kt, 0:1])
                nc.vector.tensor_copy(va[:, :, 64:65], keep_f[:, kt, :])

            # attention for (qb0, hp0) over this key block
            for t in range(4):
                attn_kt(0, 0, kb * 4 + t)
        attn_norm(0, 0)

        # ---- remaining attention combos ---------------------------------
        for qb, hp in [(0, 1), (1, 0), (1, 1)]:
            open_oP(qb, hp)
            for kt in range(KT):
                attn_kt(qb, hp, kt)
            attn_norm(qb, hp)

        # ---- output projection ------------------------------------------
        for qt in range(8):
            for eb in range(2):
                op = psA.tile([128, 512], dt.float32, tag="psA",
                              name=f"op{qt}_{eb}")
                for cc in range(2):
                    nc.tensor.matmul(op[:, :], onT_sb[cc][:, ts(qt, 128)],
                                     wo_r[cc][:, ts(eb, 512)],
                                     start=(cc == 0), stop=(cc == 1))
                osb = outsb_pool.tile([128, 512], dt.float32, tag="osb",
                                      name=f"osb{qt}_{eb}")
                nc.vector.tensor_copy(osb[:], op[:])
                nc.sync.dma_start(out[ts(qt, 128), ts(eb, 512)], osb[:])

        lp.__exit__(None, None, None)


def kernel(x_q, x_kv, pad_mask, Wq, bq, Wk, bk, Wv, bv, Wo, bo):
    global LAST_EXEC_NS
    x_q = np.asarray(x_q, np.float32)
    x_kv = np.asarray(x_kv, np.float32)
    pad_mask = np.asarray(pad_mask)
    Wq, bq = np.asarray(Wq, np.float32), np.asarray(bq, np.float32)
    Wk, bk = np.asarray(Wk, np.float32), np.asarray(bk, np.float32)
    Wv, bv = np.asarray(Wv, np.float32), np.asarray(bv, np.float32)
    Wo, bo = np.asarray(Wo, np.float32), np.asarray(bo, np.float32)

    if "nc" not in _cache:
        _cache["nc"] = _build()
    nc = _cache["nc"]

    xTq = [np.ascontiguousarray(x_q[b].T) for b in range(B)]
    xTkv = [np.ascontiguousarray(x_kv[b].T) for b in range(B)]
    keepm = []
    for b in range(B):
        k01 = (~pad_mask[b]).astype(np.float32)          # (L,) 1=keep
        k4 = np.repeat(k01[:, None], HC, axis=1)          # (L, HC)
        keepm.append(np.ascontiguousarray(
            k4.reshape(KT, 128, HC).transpose(1, 0, 2)))  # (128, KT, HC)

    in_maps = []
    for c in range(N_CORES):
        b, g = c // 4, c % 4
        hs = g * CS
        in_maps.append({
            "xTq": xTq[b],
            "xTkv": xTkv[b],
            "wq": np.ascontiguousarray(Wq[:, hs:hs + CS]),
            "wk": np.ascontiguousarray(Wk[:, hs:hs + CS]),
            "wv": np.ascontiguousarray(Wv[:, hs:hs + CS]),
            "wo": np.ascontiguousarray(Wo[hs:hs + CS, :]),
            "bqv": np.ascontiguousarray(bq[hs:hs + CS][None, :]),
            "bkv": np.ascontiguousarray(bk[hs:hs + CS][None, :]),
            "bvv": np.ascontiguousarray(bv[hs:hs + CS][None, :]),
            "keep": keepm[b],
        })

    res = run_bass_kernel_spmd(nc, in_maps, list(range(N_CORES)), trace=TRACE)
    LAST_EXEC_NS = res.exec_time_ns

    outp = np.zeros((B, N, D), np.float32)
    for c in range(N_CORES):
        outp[c // 4] += res.results[c]["out"]
    outp += bo
    return outp
